# revision 39
# baseline (speedup 1.0000x reference)
"""BailingMoE linear attention (lightning attention) on 8 trn2 NeuronCores.

Tensor-parallel over heads: 2 heads per core. Full inputs in, full output out.
Per core: qkv+g projections in fp16 (q,k,g in [chan,seq] layout, v in
[seq,chan]), per-head RMSNorm + neox RoPE (half-swap via permutation matmul),
chunked linear attention with decayed fp16 kv state in SBUF, group RMSNorm +
sigmoid gate, AllToAll (fp16, two sequence halves fired as soon as their last
producer group finishes) to sequence-shard y, then the dense projection for
the core's 1024-row output shard, overlapped with the second AllToAll.
"""
import math

import numpy as np

S = 8192
HID = 2048
H = 16
D = 128
BLK = 256
GROUPS = 8
EPS = 1e-5
ROPE_THETA = 600000.0
SCALE = D ** -0.5
N_CORES = 8
HPC = H // N_CORES          # heads per core = 2
CPC = HPC * D               # channels per core = 256
KT = 16                     # contraction tiles (2048 hid)
SEQ_G = 512                 # seq per projection group
NG = S // SEQ_G             # 16 groups
SB = S // N_CORES           # seq block per core after AllToAll = 1024
CPG = SEQ_G // BLK          # chunks per group = 2

_cache = {}


def _build_slopes():
    start = 2.0 ** (-(2.0 ** (-(math.log2(H) - 3.0))))
    slopes = np.array([start * start ** i for i in range(H)], dtype=np.float32)
    return slopes * np.float32(1.0 - 0.0 / (20 - 1) + 1e-5)


def _build_program():
    import concourse.bacc as bacc
    import concourse.tile as tile
    import concourse.mybir as mybir
    import concourse.bass_isa as bass_isa
    from contextlib import ExitStack

    dt = mybir.dt
    AF = mybir.ActivationFunctionType
    OP = mybir.AluOpType

    nc = bacc.Bacc("TRN2", target_bir_lowering=False, debug=False,
                   num_devices=N_CORES)

    def din(name, shape, dtype=dt.float16):
        return nc.dram_tensor(name, shape, dtype, kind="ExternalInput").ap()

    hsT = din("hsT", [HID, S])
    wB = din("wB", [HID, 768])          # cols: q(256) k(256) g(256)
    wv = din("wv", [HID, 256])
    dwT = din("dwT", [HID, HID])
    cosf = din("cosf", [128, S])        # [cos; cos]
    sinf = din("sinf", [128, S])        # [-sin; sin]
    qdec_d = din("qdec", [128, HPC, BLK])
    kdec_d = din("kdec", [128, HPC, BLK])
    diag_d = din("diagT", [128, HPC, 2, BLK])
    qkb_d = din("qkb", [128, 4], dt.float32)      # q0 q1 k0 k1 biases
    vb_d = din("vb", [128, 256])                  # v bias bcast over partitions
    qnw_d = din("qnw", [128, 1], dt.float32)
    knw_d = din("knw", [128, 1], dt.float32)
    gnw_d = din("gnw", [128, HPC], dt.float32)
    blk_d = din("blkdec", [128, HPC], dt.float32)
    ones128_d = din("ones128", [128, 1])
    onesr_d = din("onesr", [1, 128])    # bcast lhsT, value 1
    idm_d = din("idm", [128, 128])
    idsw_d = din("idsw", [128, 128])    # half-swap permutation
    zkv_d = din("zkv", [128, 128])      # zeros for kv init

    out_d = nc.dram_tensor("out", [SB, HID], dt.float32,
                           kind="ExternalOutput").ap()

    with tile.TileContext(nc) as tc:
        ctx = ExitStack()
        consts = ctx.enter_context(tc.tile_pool(name="consts", bufs=1))
        wpool = ctx.enter_context(tc.tile_pool(name="wpool", bufs=1))
        dramp = ctx.enter_context(tc.tile_pool(name="dramp", bufs=1,
                                               space="DRAM"))
        y_send = [dramp.tile([N_CORES, CPC, SB // 2], dt.float16,
                             name=f"y_send{i}", tag=f"y_send{i}")
                  for i in range(2)]
        y_recv = [dramp.tile([N_CORES, CPC, SB // 2], dt.float16,
                             name=f"y_recv{i}", tag=f"y_recv{i}")
                  for i in range(2)]

        def cload(name, ap_src, shape, dtype=dt.float16):
            t = consts.tile(shape, dtype, name=name, tag=name)
            nc.sync.dma_start(out=t[:], in_=ap_src)
            return t

        # small consts needed by the first group's evictions go first
        qkb = cload("qkb_s", qkb_d[:], [128, 4], dt.float32)
        qnws = cload("qnw_s", qnw_d[:], [128, 1], dt.float32)  # pre-scaled
        knw = cload("knw_s", knw_d[:], [128, 1], dt.float32)
        ones128 = cload("ones128_s", ones128_d[:], [128, 1])
        onesr = cload("onesr_s", onesr_d[:], [1, 128])
        epsb = consts.tile([128, 1], dt.float32, name="epsb", tag="epsb")
        nc.vector.memset(epsb[:], EPS)

        hsT_r = hsT.rearrange("(t p) s -> p t s", p=128)
        wB_r = wB.rearrange("(t p) c -> p t c", p=128)
        dwT_r = dwT.rearrange("(t p) c -> p t c", p=128)

        hkp = ctx.enter_context(tc.tile_pool(name="hkp", bufs=2))
        # interleave the first group's hk with wB so matmul t=0 starts asap
        wB_sb = wpool.tile([128, KT, 768], dt.float16, name="wB_sb")
        hk0 = hkp.tile([128, KT, SEQ_G], dt.float16, name="hk0", tag="hk")
        for tq in range(8):
            nc.sync.dma_start(out=wB_sb[:, tq * 2:(tq + 1) * 2, :],
                              in_=wB_r[:, tq * 2:(tq + 1) * 2, :])
            nc.sync.dma_start(out=hk0[:, tq * 2:(tq + 1) * 2, :],
                              in_=hsT_r[:, tq * 2:(tq + 1) * 2, 0:SEQ_G])
        wv_sb = wpool.tile([128, KT, 256], dt.float16, name="wv_sb")
        nc.sync.dma_start(
            out=wv_sb[:], in_=wv.rearrange("(t p) c -> p t c", p=128))
        # remaining consts (needed ~20us in, after first projections)
        idm = cload("idm_s", idm_d[:], [128, 128])
        idsw = cload("idsw_s", idsw_d[:], [128, 128])
        qdec = cload("qdec_s", qdec_d[:], [128, HPC, BLK])
        kdec = cload("kdec_s", kdec_d[:], [128, HPC, BLK])
        diag = cload("diag_s", diag_d[:], [128, HPC, 2, BLK])
        vbias = cload("vb_s", vb_d[:], [128, 256])
        gnw = cload("gnw_s", gnw_d[:], [128, HPC], dt.float32)
        blkd = cload("blkd_s", blk_d[:], [128, HPC], dt.float32)
        tabp = ctx.enter_context(tc.tile_pool(name="tabp", bufs=2))
        evp = ctx.enter_context(tc.tile_pool(name="evp", bufs=2))
        xrp = ctx.enter_context(tc.tile_pool(name="xrp", bufs=2))
        natp = ctx.enter_context(tc.tile_pool(name="natp", bufs=2))
        attp = ctx.enter_context(tc.tile_pool(name="attp", bufs=2))
        kvpl = ctx.enter_context(tc.tile_pool(name="kvpl", bufs=1))
        yp = ctx.enter_context(tc.tile_pool(name="yp", bufs=3))
        denp = ctx.enter_context(tc.tile_pool(name="denp", bufs=2))
        outp = ctx.enter_context(tc.tile_pool(name="outp", bufs=3))
        psp = ctx.enter_context(tc.tile_pool(name="psp", bufs=3, space="PSUM"))
        pse = ctx.enter_context(tc.tile_pool(name="pse", bufs=2, space="PSUM"))
        psa = ctx.enter_context(tc.tile_pool(name="psa", bufs=3, space="PSUM"))

        # dense runs 8 passes (4 col-quarters x 2 halves); each pass gets its
        # own dwq tile rotating through 2 buffers. Loads are paced: the first
        # two are issued mid main-loop (g==2, off the startup critical path),
        # load[i] thereafter is issued once pass[i-2] has consumed its tile.
        dense_order = [(0, 0), (1, 0), (2, 0), (3, 0),
                       (0, 1), (1, 1), (2, 1), (3, 1)]
        dwq_tiles = [denp.tile([128, KT, 512], dt.float16, name=f"dwq{i}",
                               tag="dwq") for i in range(8)]

        def load_dwq(i):
            cq = dense_order[i][0]
            nc.sync.dma_start(out=dwq_tiles[i][:],
                              in_=dwT_r[:, :, cq * 512:(cq + 1) * 512])

        # persistent kv state, ping-pong per head (fp16)
        kv_sb = [[kvpl.tile([128, 128], dt.float16, name=f"kv{h}_{i}",
                            tag=f"kv{h}_{i}") for i in range(2)]
                 for h in range(HPC)]
        for h in range(HPC):
            nc.sync.dma_start(out=kv_sb[h][0][:], in_=zkv_d[:])

        hk_tiles = [None] * NG
        ysb = [None, None]

        def load_group(g):
            s0 = g * SEQ_G
            if g == 0:
                hk = hk0
            else:
                hk = hkp.tile([128, KT, SEQ_G], dt.float16,
                              name=f"hk{g}", tag="hk")
                nc.sync.dma_start(out=hk[:], in_=hsT_r[:, :, s0:s0 + SEQ_G])
            hk_tiles[g] = hk
            cos_g = tabp.tile([128, SEQ_G], dt.float16, name=f"cos{g}",
                              tag="cos")
            nc.sync.dma_start(out=cos_g[:], in_=cosf[:, s0:s0 + SEQ_G])
            sin_g = tabp.tile([128, SEQ_G], dt.float16, name=f"sin{g}",
                              tag="sin")
            nc.sync.dma_start(out=sin_g[:], in_=sinf[:, s0:s0 + SEQ_G])
            return cos_g, sin_g

        nxt = load_group(0)

        for g in range(NG):
            cos_g, sin_g = nxt
            hk = hk_tiles[g]
            if g + 1 < NG:
                nxt = load_group(g + 1)
            hk_tiles[g - 1] = None

            # ---- projection accumulations ------------------------------
            xbs = []
            sig_t = []
            for ci in range(6):  # 0,1=q  2,3=k  4,5=g
                acc = psp.tile([128, SEQ_G], dt.float32,
                               name=f"acc{g}_{ci}", tag="ps")
                for t in range(KT):
                    nc.tensor.matmul(acc[:], wB_sb[:, t, ci * 128:(ci + 1) * 128],
                                     hk[:, t, :], start=(t == 0),
                                     stop=(t == KT - 1))
                if ci < 4:
                    xb = evp.tile([128, SEQ_G], dt.float16,
                                  name=f"xb{g}_{ci}", tag="xb", bufs=5)
                    nc.scalar.activation(xb[:], acc[:], AF.Identity,
                                         bias=qkb[:, ci:ci + 1])
                    xbs.append(xb)
                else:
                    # sigmoid(g) = 0.5*tanh(g/2) + 0.5 (tanh stays in the
                    # exp_and_others act table; Sigmoid would thrash tables)
                    th = evp.tile([128, SEQ_G], dt.float16,
                                  name=f"th{g}_{ci}", tag="th")
                    nc.scalar.activation(th[:], acc[:], AF.Tanh, scale=0.5)
                    sig = xrp.tile([128, SEQ_G], dt.float16,
                                   name=f"sig{g}_{ci}", tag=f"sig{ci - 4}")
                    nc.vector.tensor_scalar(out=sig[:], in0=th[:],
                                            scalar1=0.5, scalar2=0.5,
                                            op0=OP.mult, op1=OP.add)
                    sig_t.append(sig)
            v_nat = []
            for s2 in range(2):
                accv = psp.tile([128, SEQ_G], dt.float32,
                                name=f"accv{g}_{s2}", tag="ps")
                for half in range(2):
                    st = s2 * 2 + half
                    for t in range(KT):
                        nc.tensor.matmul(
                            accv[:, half * 256:(half + 1) * 256],
                            hk[:, t, st * 128:(st + 1) * 128],
                            wv_sb[:, t, :],
                            start=(t == 0), stop=(t == KT - 1))
                for half in range(2):
                    st = s2 * 2 + half
                    vn = natp.tile([128, 256], dt.float16,
                                   name=f"vn{g}_{st}", tag="vn", bufs=8)
                    nc.vector.tensor_tensor(
                        out=vn[:], in0=accv[:, half * 256:(half + 1) * 256],
                        in1=vbias[:], op=OP.add)
                    v_nat.append(vn)

            # ---- q/k norm + rope ---------------------------------------
            qr_t, kr_t = [], []
            for ci in range(4):
                is_q = ci < 2
                xb = xbs[ci]
                sq = evp.tile([128, SEQ_G], dt.float16,
                              name=f"sq{g}_{ci}", tag="sq")
                nc.vector.tensor_tensor(out=sq[:], in0=xb[:], in1=xb[:],
                                        op=OP.mult)
                ssq = pse.tile([1, SEQ_G], dt.float32,
                               name=f"ssq{g}_{ci}", tag="pse")
                nc.tensor.matmul(ssq[:], ones128[:], sq[:],
                                 start=True, stop=True)
                srt = evp.tile([1, SEQ_G], dt.float32,
                               name=f"srt{g}_{ci}", tag="srt", bufs=1)
                nc.scalar.activation(srt[:], ssq[:], AF.Sqrt,
                                     bias=epsb[0:1, :], scale=1.0 / D)
                rcp = evp.tile([1, SEQ_G], dt.float32,
                               name=f"rcp{g}_{ci}", tag="rcp", bufs=1)
                nc.vector.reciprocal_approx_fast(out=rcp[:], in_=srt[:])
                rstd = evp.tile([1, SEQ_G], dt.float16,
                                name=f"rstd{g}_{ci}", tag="rstd")
                nc.vector.tensor_copy(rstd[:], rcp[:])
                bc = pse.tile([128, SEQ_G], dt.float32,
                              name=f"bc{g}_{ci}", tag="pse")
                nc.tensor.matmul(bc[:], onesr[:], rstd[:],
                                 start=True, stop=True)
                xn = evp.tile([128, SEQ_G], dt.float16,
                              name=f"xn{g}_{ci}", tag="xn")
                nc.vector.scalar_tensor_tensor(
                    out=xn[:], in0=xb[:], scalar=qnws[:] if is_q else knw[:],
                    in1=bc[:], op0=OP.mult, op1=OP.mult)
                # rope: xr = xn*cos + swap(xn)*sin_signed
                m1 = evp.tile([128, SEQ_G], dt.float16,
                              name=f"m1{g}_{ci}", tag="m1")
                nc.vector.tensor_tensor(out=m1[:], in0=xn[:], in1=cos_g[:],
                                        op=OP.mult)
                swp = pse.tile([128, SEQ_G], dt.float32,
                               name=f"swp{g}_{ci}", tag="pse")
                nc.tensor.matmul(swp[:], idsw[:], xn[:],
                                 start=True, stop=True)
                m2 = evp.tile([128, SEQ_G], dt.float16,
                              name=f"m2{g}_{ci}", tag="m2")
                nc.vector.tensor_tensor(out=m2[:], in0=swp[:], in1=sin_g[:],
                                        op=OP.mult)
                xr = xrp.tile([128, SEQ_G], dt.float16,
                              name=f"xr{g}_{ci}", tag=f"xr{ci}")
                nc.vector.tensor_tensor(out=xr[:], in0=m1[:], in1=m2[:],
                                        op=OP.add)
                if is_q:
                    qr_t.append(xr)
                else:
                    kr_t.append(xr)

            # decayed k for the kv update, per chunk; then natural layout
            knat = [[None] * CPG for _ in range(HPC)]
            for h in range(HPC):
                ktil = xrp.tile([128, SEQ_G], dt.float16,
                                name=f"ktil{g}_{h}", tag=f"ktil{h}")
                for cc in range(CPG):
                    nc.vector.tensor_tensor(
                        out=ktil[:, cc * BLK:(cc + 1) * BLK],
                        in0=kr_t[h][:, cc * BLK:(cc + 1) * BLK],
                        in1=kdec[:, h, :], op=OP.mult)
                for cc in range(CPG):
                    kn_list = []
                    for j in range(2):
                        tp = pse.tile([128, 128], dt.float16,
                                      name=f"tp{g}_{h}_{cc}_{j}", tag="pse")
                        nc.tensor.transpose(
                            tp[:],
                            ktil[:, cc * BLK + j * 128:cc * BLK + (j + 1) * 128],
                            idm[:])
                        kn = natp.tile([128, 128], dt.float16,
                                       name=f"kn{g}_{h}_{cc}_{j}", tag="kn",
                                       bufs=16)
                        nc.scalar.activation(kn[:], tp[:], AF.Copy)
                        kn_list.append(kn)
                    knat[h][cc] = kn_list

            # ---- attention chunks --------------------------------------
            for cc in range(CPG):
                ch = g * CPG + cc
                # half = first/second half of the SEQUENCE (so AllToAll #1
                # fires at group 7); core b's shard is rows
                # [4096*half + 512*b, +512) — reassembled on the host.
                half = ch // 16
                b = (ch % 16) // 2
                off = (ch % 2) * BLK
                o_ps = []
                sqs = []
                for h in range(HPC):
                    qr = qr_t[h][:, cc * BLK:(cc + 1) * BLK]
                    kv_cur = kv_sb[h][ch % 2]
                    kv_nxt = kv_sb[h][(ch + 1) % 2]
                    # kq[j, i] masked
                    kq = psa.tile([128, SEQ_G], dt.float32,
                                  name=f"kq{ch}_{h}", tag="psa")
                    kqd = []
                    for j in range(2):
                        nc.tensor.matmul(
                            kq[:, j * BLK:(j + 1) * BLK],
                            kr_t[h][:, cc * BLK + j * 128:cc * BLK + (j + 1) * 128],
                            qr, start=True, stop=True)
                        kqj = attp.tile([128, BLK], dt.float16,
                                        name=f"kqd{ch}_{h}_{j}", tag="kqd",
                                        bufs=3)
                        nc.vector.tensor_tensor(
                            out=kqj[:], in0=kq[:, j * BLK:(j + 1) * BLK],
                            in1=diag[:, h, j, :], op=OP.mult)
                        kqd.append(kqj)
                    # q with decay
                    qt = attp.tile([128, BLK], dt.float16,
                                   name=f"qt{ch}_{h}", tag="qt")
                    nc.vector.tensor_tensor(out=qt[:], in0=qr,
                                            in1=qdec[:, h, :], op=OP.mult)
                    # outT = v0.T@kqd0 + v1.T@kqd1 + kv.T@qt
                    ops = psa.tile([128, BLK], dt.float32,
                                   name=f"ops{ch}_{h}", tag="psa")
                    for j in range(2):
                        nc.tensor.matmul(
                            ops[:], v_nat[cc * 2 + j][:, h * 128:(h + 1) * 128],
                            kqd[j][:], start=(j == 0), stop=False)
                    nc.tensor.matmul(ops[:], kv_cur[:], qt[:],
                                     start=False, stop=True)
                    o_ps.append(ops)
                    # kv update
                    kvp_ps = psa.tile([128, 128], dt.float32,
                                      name=f"kvp{ch}_{h}", tag="psa")
                    for j in range(2):
                        nc.tensor.matmul(
                            kvp_ps[:], knat[h][cc][j][:],
                            v_nat[cc * 2 + j][:, h * 128:(h + 1) * 128],
                            start=(j == 0), stop=(j == 1))
                    nc.vector.scalar_tensor_tensor(
                        out=kv_nxt[:], in0=kv_cur[:], scalar=blkd[:, h:h + 1],
                        in1=kvp_ps[:], op0=OP.mult, op1=OP.add)
                    sqh = attp.tile([128, BLK], dt.float16,
                                    name=f"gsq{ch}_{h}", tag="gsq", bufs=2)
                    nc.scalar.activation(sqh[:], ops[:], AF.Square)
                    sqs.append(sqh)

                # group rmsnorm over both heads + gate
                gs = []
                for h in range(HPC):
                    s = attp.tile([128, BLK], dt.float32,
                                  name=f"gs{ch}_{h}", tag=f"gs{h}", bufs=1)
                    nc.gpsimd.partition_all_reduce(s[:], sqs[h][:], 128,
                                                   bass_isa.ReduceOp.add)
                    gs.append(s)
                gsum = attp.tile([128, BLK], dt.float32,
                                 name=f"gsum{ch}", tag="gsum")
                nc.vector.tensor_tensor(out=gsum[:], in0=gs[0][:],
                                        in1=gs[1][:], op=OP.add)
                gsrt = attp.tile([128, BLK], dt.float32,
                                 name=f"gsrt{ch}", tag="gsrt", bufs=1)
                nc.scalar.activation(gsrt[:], gsum[:], AF.Sqrt,
                                     bias=epsb[:], scale=1.0 / CPC)
                grcp = attp.tile([128, BLK], dt.float32,
                                 name=f"grcp{ch}", tag="grcp")
                nc.vector.reciprocal_approx_fast(out=grcp[:], in_=gsrt[:])
                y2 = yp.tile([128, HPC, BLK], dt.float16,
                             name=f"y2{ch}", tag="y2")
                for h in range(HPC):
                    y1 = yp.tile([128, BLK], dt.float16,
                                 name=f"y1{ch}_{h}", tag="y1")
                    nc.vector.scalar_tensor_tensor(
                        out=y1[:], in0=o_ps[h][:], scalar=gnw[:, h:h + 1],
                        in1=grcp[:], op0=OP.mult, op1=OP.mult)
                    nc.vector.tensor_tensor(
                        out=y2[:, h, :], in0=y1[:],
                        in1=sig_t[h][:, cc * BLK:(cc + 1) * BLK], op=OP.mult)
                nc.sync.dma_start(
                    out=y_send[half][b].rearrange(
                        "(h p) s -> p h s", p=128)[:, :, off:off + BLK],
                    in_=y2[:])

            if g == 2:
                load_dwq(0)
                load_dwq(1)
            # ---- fire AllToAll as soon as a seq-half is complete -------
            if g == 7 or g == NG - 1:
                hf = 0 if g == 7 else 1
                nc.gpsimd.collective_compute(
                    "AllToAll", mybir.AluOpType.bypass,
                    replica_groups=[list(range(N_CORES))],
                    ins=[y_send[hf][:].opt()],
                    outs=[y_recv[hf][:].opt()],
                )
                yh = denp.tile([128, KT, 512], dt.float16, name=f"ysb{hf}",
                               tag="ysb")
                yr = y_recv[hf][:].rearrange("b (t p) s -> p (b t) s", p=128)
                for sb_i in range(4):
                    nc.gpsimd.dma_start(
                        out=yh[:, :, sb_i * 128:(sb_i + 1) * 128],
                        in_=yr[:, :, sb_i * 128:(sb_i + 1) * 128])
                ysb[hf] = yh

        # ---- dense ------------------------------------------------------
        # half 0 first (its AllToAll finished back at group 7); the four
        # half-0 passes hide AllToAll #2 + the ysb1 load.
        for i, (cq, half) in enumerate(dense_order):
            for sb_i in range(4):
                acc = psp.tile([128, 512], dt.float32,
                               name=f"dacc{cq}_{half}_{sb_i}", tag="ps")
                for t in range(KT):
                    nc.tensor.matmul(
                        acc[:], ysb[half][:, t, sb_i * 128:(sb_i + 1) * 128],
                        dwq_tiles[i][:, t, :],
                        start=(t == 0), stop=(t == KT - 1))
                ot = outp.tile([128, 512], dt.float32,
                               name=f"ot{cq}_{half}_{sb_i}", tag="ot")
                nc.scalar.activation(ot[:], acc[:], AF.Copy)
                srow = half * 512 + sb_i * 128
                nc.sync.dma_start(
                    out=out_d[srow:srow + 128, cq * 512:(cq + 1) * 512],
                    in_=ot[:])
            if i + 2 < len(dense_order):
                load_dwq(i + 2)
        ctx.close()

    nc.compile()
    return nc


def _stage(hidden_states, positions, qkv_w, qkv_b, q_norm_w, k_norm_w,
           g_w, g_norm_w, dense_w):
    f32 = np.float32
    f16 = np.float16
    hidden_states = np.asarray(hidden_states, dtype=f32)
    positions = np.asarray(positions)
    qkv_w = np.asarray(qkv_w, dtype=f32)
    qkv_b = np.asarray(qkv_b, dtype=f32)
    q_norm_w = np.asarray(q_norm_w, dtype=f32)
    k_norm_w = np.asarray(k_norm_w, dtype=f32)
    g_w = np.asarray(g_w, dtype=f32)
    g_norm_w = np.asarray(g_norm_w, dtype=f32)
    dense_w = np.asarray(dense_w, dtype=f32)
    slopes = _build_slopes()

    hsT = np.ascontiguousarray(hidden_states.T).astype(f16)

    inv_freq = 1.0 / (ROPE_THETA ** (np.arange(0, D, 2, dtype=f32) / D))
    freqs = positions.astype(f32)[:, None] * inv_freq[None, :]  # [S, 64]
    cos = np.cos(freqs).T.astype(f32)     # [64, S]
    sin = np.sin(freqs).T.astype(f32)
    cosf = np.ascontiguousarray(np.concatenate([cos, cos], axis=0)).astype(f16)
    sinf = np.ascontiguousarray(np.concatenate([-sin, sin], axis=0)).astype(f16)

    idx = np.arange(BLK, dtype=f32)
    dwT = np.ascontiguousarray(dense_w.T).astype(f16)
    ones128 = np.ones((128, 1), dtype=f16)
    onesr = np.ones((1, 128), dtype=f16)
    idm = np.eye(128, dtype=f16)
    idsw = np.zeros((128, 128), dtype=f16)
    for m in range(128):
        idsw[(m + 64) % 128, m] = 1.0
    qnw = (q_norm_w.reshape(128, 1) * SCALE).astype(f32)  # SCALE folded in
    knw = k_norm_w.reshape(128, 1).astype(f32).copy()

    in_maps = []
    for j in range(N_CORES):
        heads = [j * HPC + h for h in range(HPC)]
        c0 = j * CPC
        wBm = np.zeros((HID, 768), dtype=f16)
        wBm[:, 0:256] = qkv_w[c0:c0 + CPC, :].T.astype(f16)
        wBm[:, 256:512] = qkv_w[HID + c0:HID + c0 + CPC, :].T.astype(f16)
        wBm[:, 512:768] = g_w[c0:c0 + CPC, :].T.astype(f16)
        wvm = qkv_w[2 * HID + c0:2 * HID + c0 + CPC, :].T.astype(f16)
        wvm = np.ascontiguousarray(wvm)

        qb = qkv_b[c0:c0 + CPC].reshape(HPC, 128).T       # [128, 2]
        kb = qkv_b[HID + c0:HID + c0 + CPC].reshape(HPC, 128).T
        qkbm = np.ascontiguousarray(
            np.concatenate([qb, kb], axis=1)).astype(f32)  # [128, 4]
        vb = qkv_b[2 * HID + c0:2 * HID + c0 + CPC].astype(f16)  # [256]
        vbm = np.ascontiguousarray(np.broadcast_to(vb[None, :], (128, 256)))

        sl = slopes[heads]  # [HPC]
        qdec = np.exp(-sl[:, None] * (idx + 1.0)[None, :]).astype(f16)
        qdec = np.ascontiguousarray(
            np.broadcast_to(qdec[None, :, :], (128, HPC, BLK)))
        kd = np.exp(-sl[:, None] * (BLK - 1.0 - idx)[None, :]).astype(f16)
        kdecm = np.ascontiguousarray(
            np.broadcast_to(kd[None, :, :], (128, HPC, BLK)))
        dif = idx[:, None] - idx[None, :]           # [i, j]
        diagT = np.zeros((128, HPC, 2, BLK), dtype=f16)
        for hh in range(HPC):
            dd = np.where(
                dif >= 0,
                np.exp(-sl[hh] * np.where(dif >= 0, dif, 0.0)),
                0.0).astype(f16)                    # [i, j]
            ddT = dd.T                               # [j, i]
            diagT[:, hh, 0, :] = ddT[0:128]
            diagT[:, hh, 1, :] = ddT[128:256]
        blkdec = np.ascontiguousarray(np.broadcast_to(
            np.exp(-sl * BLK).astype(f32)[None, :], (128, HPC)))
        gnwm = np.ascontiguousarray(
            g_norm_w[c0:c0 + CPC].reshape(HPC, 128).T).astype(f32)

        in_maps.append({
            "hsT": hsT, "wB": wBm, "wv": wvm, "dwT": dwT,
            "cosf": cosf, "sinf": sinf,
            "qdec": qdec, "kdec": kdecm, "diagT": diagT,
            "qkb": qkbm, "vb": vbm,
            "qnw": qnw, "knw": knw, "gnw": gnwm, "blkdec": blkdec,
            "ones128": ones128, "onesr": onesr,
            "idm": idm, "idsw": idsw,
            "zkv": np.zeros((128, 128), dtype=f16),
        })
    return in_maps


def kernel(**inputs):
    from concourse.bass_utils import run_bass_kernel_spmd

    if "nc" not in _cache:
        _cache["nc"] = _build_program()
    nc = _cache["nc"]
    in_maps = _stage(**inputs)
    res = run_bass_kernel_spmd(nc, in_maps, list(range(N_CORES)))
    return _assemble(res)


def _assemble(res):
    # core j's shard is rows [4096*p + 512*j, +512) for p in {0,1}
    out = np.empty((S, HID), dtype=np.float32)
    for j in range(N_CORES):
        rj = res.results[j]["out"]
        for p in range(2):
            out[4096 * p + 512 * j:4096 * p + 512 * (j + 1)] = \
                rj[512 * p:512 * (p + 1)]
    return out


# revision 40
# speedup vs baseline: 1.0728x; 1.0728x over previous
"""BailingMoE linear attention (lightning attention) on 8 trn2 NeuronCores.

Tensor-parallel over heads: 2 heads per core. Full inputs in, full output out.
Per core: qkv+g projections in fp16 (q,k,g in [chan,seq] layout, v in
[seq,chan]), per-head RMSNorm + neox RoPE (half-swap via permutation matmul),
chunked linear attention with decayed fp16 kv state in SBUF, group RMSNorm +
sigmoid gate, AllToAll (fp16, two sequence halves fired as soon as their last
producer group finishes) to sequence-shard y, then the dense projection for
the core's 1024-row output shard, overlapped with the second AllToAll.
"""
import math

import numpy as np

S = 8192
HID = 2048
H = 16
D = 128
BLK = 256
GROUPS = 8
EPS = 1e-5
ROPE_THETA = 600000.0
SCALE = D ** -0.5
N_CORES = 8
HPC = H // N_CORES          # heads per core = 2
CPC = HPC * D               # channels per core = 256
KT = 16                     # contraction tiles (2048 hid)
SEQ_G = 512                 # seq per projection group
NG = S // SEQ_G             # 16 groups
SB = S // N_CORES           # seq block per core after AllToAll = 1024
CPG = SEQ_G // BLK          # chunks per group = 2

_cache = {}


def _build_slopes():
    start = 2.0 ** (-(2.0 ** (-(math.log2(H) - 3.0))))
    slopes = np.array([start * start ** i for i in range(H)], dtype=np.float32)
    return slopes * np.float32(1.0 - 0.0 / (20 - 1) + 1e-5)


def _build_program():
    import concourse.bacc as bacc
    import concourse.tile as tile
    import concourse.mybir as mybir
    import concourse.bass_isa as bass_isa
    from contextlib import ExitStack

    dt = mybir.dt
    AF = mybir.ActivationFunctionType
    OP = mybir.AluOpType

    nc = bacc.Bacc("TRN2", target_bir_lowering=False, debug=False,
                   num_devices=N_CORES)

    def din(name, shape, dtype=dt.float16):
        return nc.dram_tensor(name, shape, dtype, kind="ExternalInput").ap()

    hsT = din("hsT", [HID, S])
    wB = din("wB", [HID, 768])          # cols: q(256) k(256) g(256)
    wv = din("wv", [HID, 256])
    dwT = din("dwT", [HID, HID])
    cosf = din("cosf", [128, S])        # [cos; cos]
    sinf = din("sinf", [128, S])        # [-sin; sin]
    qdec_d = din("qdec", [128, HPC, BLK])
    kdec_d = din("kdec", [128, HPC, BLK])
    diag_d = din("diagT", [128, HPC, 2, BLK])
    qkb_d = din("qkb", [128, 4], dt.float32)      # q0 q1 k0 k1 biases
    vb_d = din("vb", [128, 256])                  # v bias bcast over partitions
    qnw_d = din("qnw", [128, 1], dt.float32)
    knw_d = din("knw", [128, 1], dt.float32)
    gnw_d = din("gnw", [128, HPC], dt.float32)
    blk_d = din("blkdec", [128, HPC], dt.float32)
    ones128_d = din("ones128", [128, 1])
    onesr_d = din("onesr", [1, 128])    # bcast lhsT, value 1
    idm_d = din("idm", [128, 128])
    idsw_d = din("idsw", [128, 128])    # half-swap permutation
    zkv_d = din("zkv", [128, 128])      # zeros for kv init

    out_d = nc.dram_tensor("out", [SB, HID], dt.float32,
                           kind="ExternalOutput").ap()

    with tile.TileContext(nc) as tc:
        ctx = ExitStack()
        consts = ctx.enter_context(tc.tile_pool(name="consts", bufs=1))
        wpool = ctx.enter_context(tc.tile_pool(name="wpool", bufs=1))
        dramp = ctx.enter_context(tc.tile_pool(name="dramp", bufs=1,
                                               space="DRAM"))
        y_send = [dramp.tile([N_CORES, CPC, SB // 2], dt.float16,
                             name=f"y_send{i}", tag=f"y_send{i}")
                  for i in range(2)]
        y_recv = [dramp.tile([N_CORES, CPC, SB // 2], dt.float16,
                             name=f"y_recv{i}", tag=f"y_recv{i}")
                  for i in range(2)]

        def cload(name, ap_src, shape, dtype=dt.float16):
            t = consts.tile(shape, dtype, name=name, tag=name)
            nc.sync.dma_start(out=t[:], in_=ap_src)
            return t

        # small consts needed by the first group's evictions go first
        qkb = cload("qkb_s", qkb_d[:], [128, 4], dt.float32)
        qnws = cload("qnw_s", qnw_d[:], [128, 1], dt.float32)  # pre-scaled
        knw = cload("knw_s", knw_d[:], [128, 1], dt.float32)
        ones128 = cload("ones128_s", ones128_d[:], [128, 1])
        onesr = cload("onesr_s", onesr_d[:], [1, 128])
        epsb = consts.tile([128, 1], dt.float32, name="epsb", tag="epsb")
        nc.vector.memset(epsb[:], EPS)

        hsT_r = hsT.rearrange("(t p) s -> p t s", p=128)
        wB_r = wB.rearrange("(t p) c -> p t c", p=128)
        dwT_r = dwT.rearrange("(t p) c -> p t c", p=128)

        hkp = ctx.enter_context(tc.tile_pool(name="hkp", bufs=2))
        # interleave the first group's hk with wB so matmul t=0 starts asap
        wB_sb = wpool.tile([128, KT, 768], dt.float16, name="wB_sb")
        hk0 = hkp.tile([128, KT, SEQ_G], dt.float16, name="hk0", tag="hk")
        for tq in range(8):
            nc.sync.dma_start(out=wB_sb[:, tq * 2:(tq + 1) * 2, :],
                              in_=wB_r[:, tq * 2:(tq + 1) * 2, :])
            nc.sync.dma_start(out=hk0[:, tq * 2:(tq + 1) * 2, :],
                              in_=hsT_r[:, tq * 2:(tq + 1) * 2, 0:SEQ_G])
        wv_sb = wpool.tile([128, KT, 256], dt.float16, name="wv_sb")
        nc.sync.dma_start(
            out=wv_sb[:], in_=wv.rearrange("(t p) c -> p t c", p=128))
        # remaining consts (needed ~20us in, after first projections)
        idm = cload("idm_s", idm_d[:], [128, 128])
        idsw = cload("idsw_s", idsw_d[:], [128, 128])
        qdec = cload("qdec_s", qdec_d[:], [128, HPC, BLK])
        kdec = cload("kdec_s", kdec_d[:], [128, HPC, BLK])
        diag = cload("diag_s", diag_d[:], [128, HPC, 2, BLK])
        vbias = cload("vb_s", vb_d[:], [128, 256])
        gnw = cload("gnw_s", gnw_d[:], [128, HPC], dt.float32)
        blkd = cload("blkd_s", blk_d[:], [128, HPC], dt.float32)
        tabp = ctx.enter_context(tc.tile_pool(name="tabp", bufs=2))
        evp = ctx.enter_context(tc.tile_pool(name="evp", bufs=2))
        xrp = ctx.enter_context(tc.tile_pool(name="xrp", bufs=2))
        natp = ctx.enter_context(tc.tile_pool(name="natp", bufs=2))
        attp = ctx.enter_context(tc.tile_pool(name="attp", bufs=2))
        kvpl = ctx.enter_context(tc.tile_pool(name="kvpl", bufs=1))
        yp = ctx.enter_context(tc.tile_pool(name="yp", bufs=3))
        denp = ctx.enter_context(tc.tile_pool(name="denp", bufs=2))
        outp = ctx.enter_context(tc.tile_pool(name="outp", bufs=3))
        psp = ctx.enter_context(tc.tile_pool(name="psp", bufs=3, space="PSUM"))
        pse = ctx.enter_context(tc.tile_pool(name="pse", bufs=2, space="PSUM"))
        psa = ctx.enter_context(tc.tile_pool(name="psa", bufs=3, space="PSUM"))

        # dense runs 8 passes (4 col-quarters x 2 halves); each pass gets its
        # own dwq tile rotating through 2 buffers. Loads are paced: the first
        # two are issued mid main-loop (g==2, off the startup critical path),
        # load[i] thereafter is issued once pass[i-2] has consumed its tile.
        dense_order = [(0, 0), (1, 0), (2, 0), (3, 0),
                       (0, 1), (1, 1), (2, 1), (3, 1)]
        dwq_tiles = [denp.tile([128, KT, 512], dt.float16, name=f"dwq{i}",
                               tag="dwq") for i in range(8)]

        def load_dwq(i):
            cq = dense_order[i][0]
            nc.sync.dma_start(out=dwq_tiles[i][:],
                              in_=dwT_r[:, :, cq * 512:(cq + 1) * 512])

        # persistent kv state, ping-pong per head (fp16)
        kv_sb = [[kvpl.tile([128, 128], dt.float16, name=f"kv{h}_{i}",
                            tag=f"kv{h}_{i}") for i in range(2)]
                 for h in range(HPC)]
        for h in range(HPC):
            nc.sync.dma_start(out=kv_sb[h][0][:], in_=zkv_d[:])

        hk_tiles = [None] * NG
        ysb = [None, None]

        def load_group(g):
            s0 = g * SEQ_G
            if g == 0:
                hk = hk0
            else:
                hk = hkp.tile([128, KT, SEQ_G], dt.float16,
                              name=f"hk{g}", tag="hk")
                nc.sync.dma_start(out=hk[:], in_=hsT_r[:, :, s0:s0 + SEQ_G])
            hk_tiles[g] = hk
            cos_g = tabp.tile([128, SEQ_G], dt.float16, name=f"cos{g}",
                              tag="cos")
            nc.sync.dma_start(out=cos_g[:], in_=cosf[:, s0:s0 + SEQ_G])
            sin_g = tabp.tile([128, SEQ_G], dt.float16, name=f"sin{g}",
                              tag="sin")
            nc.sync.dma_start(out=sin_g[:], in_=sinf[:, s0:s0 + SEQ_G])
            return cos_g, sin_g

        nxt = load_group(0)

        for g in range(NG):
            cos_g, sin_g = nxt
            hk = hk_tiles[g]
            if g + 1 < NG:
                nxt = load_group(g + 1)
            hk_tiles[g - 1] = None

            # ---- projection accumulations ------------------------------
            xbs = []
            sig_t = []
            for ci in range(6):  # 0,1=q  2,3=k  4,5=g
                acc = psp.tile([128, SEQ_G], dt.float32,
                               name=f"acc{g}_{ci}", tag="ps")
                for t in range(KT):
                    nc.tensor.matmul(acc[:], wB_sb[:, t, ci * 128:(ci + 1) * 128],
                                     hk[:, t, :], start=(t == 0),
                                     stop=(t == KT - 1))
                if ci < 4:
                    xb = evp.tile([128, SEQ_G], dt.float16,
                                  name=f"xb{g}_{ci}", tag="xb", bufs=5)
                    nc.scalar.activation(xb[:], acc[:], AF.Identity,
                                         bias=qkb[:, ci:ci + 1])
                    xbs.append(xb)
                else:
                    # sigmoid(g) = 0.5*tanh(g/2) + 0.5 (tanh stays in the
                    # exp_and_others act table; Sigmoid would thrash tables)
                    th = evp.tile([128, SEQ_G], dt.float16,
                                  name=f"th{g}_{ci}", tag="th")
                    nc.scalar.activation(th[:], acc[:], AF.Tanh, scale=0.5)
                    sig = xrp.tile([128, SEQ_G], dt.float16,
                                   name=f"sig{g}_{ci}", tag=f"sig{ci - 4}")
                    nc.vector.tensor_scalar(out=sig[:], in0=th[:],
                                            scalar1=0.5, scalar2=0.5,
                                            op0=OP.mult, op1=OP.add)
                    sig_t.append(sig)
            v_nat = []
            for s2 in range(2):
                accv = psp.tile([128, SEQ_G], dt.float32,
                                name=f"accv{g}_{s2}", tag="ps")
                for half in range(2):
                    st = s2 * 2 + half
                    for t in range(KT):
                        nc.tensor.matmul(
                            accv[:, half * 256:(half + 1) * 256],
                            hk[:, t, st * 128:(st + 1) * 128],
                            wv_sb[:, t, :],
                            start=(t == 0), stop=(t == KT - 1))
                for half in range(2):
                    st = s2 * 2 + half
                    vn = natp.tile([128, 256], dt.float16,
                                   name=f"vn{g}_{st}", tag="vn", bufs=8)
                    nc.vector.tensor_tensor(
                        out=vn[:], in0=accv[:, half * 256:(half + 1) * 256],
                        in1=vbias[:], op=OP.add)
                    v_nat.append(vn)

            # ---- q/k norm + rope ---------------------------------------
            qr_t, kr_t = [], []
            for ci in range(4):
                is_q = ci < 2
                xb = xbs[ci]
                sq = evp.tile([128, SEQ_G], dt.float16,
                              name=f"sq{g}_{ci}", tag="sq")
                nc.vector.tensor_tensor(out=sq[:], in0=xb[:], in1=xb[:],
                                        op=OP.mult)
                ssq = pse.tile([1, SEQ_G], dt.float32,
                               name=f"ssq{g}_{ci}", tag="pse")
                nc.tensor.matmul(ssq[:], ones128[:], sq[:],
                                 start=True, stop=True)
                srt = evp.tile([1, SEQ_G], dt.float32,
                               name=f"srt{g}_{ci}", tag="srt", bufs=1)
                nc.scalar.activation(srt[:], ssq[:], AF.Sqrt,
                                     bias=epsb[0:1, :], scale=1.0 / D)
                rcp = evp.tile([1, SEQ_G], dt.float32,
                               name=f"rcp{g}_{ci}", tag="rcp", bufs=1)
                nc.vector.reciprocal_approx_fast(out=rcp[:], in_=srt[:])
                rstd = evp.tile([1, SEQ_G], dt.float16,
                                name=f"rstd{g}_{ci}", tag="rstd")
                nc.vector.tensor_copy(rstd[:], rcp[:])
                bc = pse.tile([128, SEQ_G], dt.float32,
                              name=f"bc{g}_{ci}", tag="pse")
                nc.tensor.matmul(bc[:], onesr[:], rstd[:],
                                 start=True, stop=True)
                xn = evp.tile([128, SEQ_G], dt.float16,
                              name=f"xn{g}_{ci}", tag="xn")
                nc.vector.scalar_tensor_tensor(
                    out=xn[:], in0=xb[:], scalar=qnws[:] if is_q else knw[:],
                    in1=bc[:], op0=OP.mult, op1=OP.mult)
                # rope: xr = xn*cos + swap(xn)*sin_signed
                m1 = evp.tile([128, SEQ_G], dt.float16,
                              name=f"m1{g}_{ci}", tag="m1")
                nc.vector.tensor_tensor(out=m1[:], in0=xn[:], in1=cos_g[:],
                                        op=OP.mult)
                swp = pse.tile([128, SEQ_G], dt.float32,
                               name=f"swp{g}_{ci}", tag="pse")
                nc.tensor.matmul(swp[:], idsw[:], xn[:],
                                 start=True, stop=True)
                m2 = evp.tile([128, SEQ_G], dt.float16,
                              name=f"m2{g}_{ci}", tag="m2")
                nc.vector.tensor_tensor(out=m2[:], in0=swp[:], in1=sin_g[:],
                                        op=OP.mult)
                xr = xrp.tile([128, SEQ_G], dt.float16,
                              name=f"xr{g}_{ci}", tag=f"xr{ci}")
                nc.vector.tensor_tensor(out=xr[:], in0=m1[:], in1=m2[:],
                                        op=OP.add)
                if is_q:
                    qr_t.append(xr)
                else:
                    kr_t.append(xr)

            # decayed k for the kv update, per chunk; then natural layout
            knat = [[None] * CPG for _ in range(HPC)]
            for h in range(HPC):
                ktil = xrp.tile([128, SEQ_G], dt.float16,
                                name=f"ktil{g}_{h}", tag=f"ktil{h}")
                for cc in range(CPG):
                    nc.vector.tensor_tensor(
                        out=ktil[:, cc * BLK:(cc + 1) * BLK],
                        in0=kr_t[h][:, cc * BLK:(cc + 1) * BLK],
                        in1=kdec[:, h, :], op=OP.mult)
                for cc in range(CPG):
                    kn_list = []
                    for j in range(2):
                        tp = pse.tile([128, 128], dt.float16,
                                      name=f"tp{g}_{h}_{cc}_{j}", tag="pse")
                        nc.tensor.transpose(
                            tp[:],
                            ktil[:, cc * BLK + j * 128:cc * BLK + (j + 1) * 128],
                            idm[:])
                        kn = natp.tile([128, 128], dt.float16,
                                       name=f"kn{g}_{h}_{cc}_{j}", tag="kn",
                                       bufs=16)
                        nc.scalar.activation(kn[:], tp[:], AF.Copy)
                        kn_list.append(kn)
                    knat[h][cc] = kn_list

            # ---- attention chunks --------------------------------------
            for cc in range(CPG):
                ch = g * CPG + cc
                # half = first/second half of the SEQUENCE (so AllToAll #1
                # fires at group 7); core b's shard is rows
                # [4096*half + 512*b, +512) — reassembled on the host.
                half = ch // 16
                b = (ch % 16) // 2
                off = (ch % 2) * BLK
                o_ps = []
                sqs = []
                for h in range(HPC):
                    qr = qr_t[h][:, cc * BLK:(cc + 1) * BLK]
                    kv_cur = kv_sb[h][ch % 2]
                    kv_nxt = kv_sb[h][(ch + 1) % 2]
                    # kq[j, i] masked
                    kq = psa.tile([128, SEQ_G], dt.float32,
                                  name=f"kq{ch}_{h}", tag="psa")
                    kqd = []
                    for j in range(2):
                        nc.tensor.matmul(
                            kq[:, j * BLK:(j + 1) * BLK],
                            kr_t[h][:, cc * BLK + j * 128:cc * BLK + (j + 1) * 128],
                            qr, start=True, stop=True)
                        kqj = attp.tile([128, BLK], dt.float16,
                                        name=f"kqd{ch}_{h}_{j}", tag="kqd",
                                        bufs=3)
                        nc.vector.tensor_tensor(
                            out=kqj[:], in0=kq[:, j * BLK:(j + 1) * BLK],
                            in1=diag[:, h, j, :], op=OP.mult)
                        kqd.append(kqj)
                    # q with decay
                    qt = attp.tile([128, BLK], dt.float16,
                                   name=f"qt{ch}_{h}", tag="qt")
                    nc.vector.tensor_tensor(out=qt[:], in0=qr,
                                            in1=qdec[:, h, :], op=OP.mult)
                    # outT = v0.T@kqd0 + v1.T@kqd1 + kv.T@qt
                    ops = psa.tile([128, BLK], dt.float32,
                                   name=f"ops{ch}_{h}", tag="psa")
                    for j in range(2):
                        nc.tensor.matmul(
                            ops[:], v_nat[cc * 2 + j][:, h * 128:(h + 1) * 128],
                            kqd[j][:], start=(j == 0), stop=False)
                    nc.tensor.matmul(ops[:], kv_cur[:], qt[:],
                                     start=False, stop=True)
                    o_ps.append(ops)
                    # kv update
                    kvp_ps = psa.tile([128, 128], dt.float32,
                                      name=f"kvp{ch}_{h}", tag="psa")
                    for j in range(2):
                        nc.tensor.matmul(
                            kvp_ps[:], knat[h][cc][j][:],
                            v_nat[cc * 2 + j][:, h * 128:(h + 1) * 128],
                            start=(j == 0), stop=(j == 1))
                    nc.vector.scalar_tensor_tensor(
                        out=kv_nxt[:], in0=kv_cur[:], scalar=blkd[:, h:h + 1],
                        in1=kvp_ps[:], op0=OP.mult, op1=OP.add)
                    sqh = attp.tile([128, BLK], dt.float16,
                                    name=f"gsq{ch}_{h}", tag="gsq", bufs=2)
                    nc.scalar.activation(sqh[:], ops[:], AF.Square)
                    sqs.append(sqh)

                # group rmsnorm over both heads + gate
                gs = []
                for h in range(HPC):
                    s = attp.tile([128, BLK], dt.float32,
                                  name=f"gs{ch}_{h}", tag=f"gs{h}", bufs=1)
                    nc.gpsimd.partition_all_reduce(s[:], sqs[h][:], 128,
                                                   bass_isa.ReduceOp.add)
                    gs.append(s)
                gsum = attp.tile([128, BLK], dt.float32,
                                 name=f"gsum{ch}", tag="gsum")
                nc.vector.tensor_tensor(out=gsum[:], in0=gs[0][:],
                                        in1=gs[1][:], op=OP.add)
                gsrt = attp.tile([128, BLK], dt.float32,
                                 name=f"gsrt{ch}", tag="gsrt", bufs=1)
                nc.scalar.activation(gsrt[:], gsum[:], AF.Sqrt,
                                     bias=epsb[:], scale=1.0 / CPC)
                grcp = attp.tile([128, BLK], dt.float32,
                                 name=f"grcp{ch}", tag="grcp")
                nc.vector.reciprocal_approx_fast(out=grcp[:], in_=gsrt[:])
                y2 = yp.tile([128, HPC, BLK], dt.float16,
                             name=f"y2{ch}", tag="y2")
                for h in range(HPC):
                    y1 = yp.tile([128, BLK], dt.float16,
                                 name=f"y1{ch}_{h}", tag="y1")
                    nc.vector.scalar_tensor_tensor(
                        out=y1[:], in0=o_ps[h][:], scalar=gnw[:, h:h + 1],
                        in1=grcp[:], op0=OP.mult, op1=OP.mult)
                    nc.vector.tensor_tensor(
                        out=y2[:, h, :], in0=y1[:],
                        in1=sig_t[h][:, cc * BLK:(cc + 1) * BLK], op=OP.mult)
                nc.sync.dma_start(
                    out=y_send[half][b].rearrange(
                        "(h p) s -> p h s", p=128)[:, :, off:off + BLK],
                    in_=y2[:])

            if g == 2:
                load_dwq(0)
                load_dwq(1)
            # ---- fire AllToAll as soon as a seq-half is complete -------
            if g == 7 or g == NG - 1:
                hf = 0 if g == 7 else 1
                nc.gpsimd.collective_compute(
                    "AllToAll", mybir.AluOpType.bypass,
                    replica_groups=[list(range(N_CORES))],
                    ins=[y_send[hf][:].opt()],
                    outs=[y_recv[hf][:].opt()],
                )
            # ysb0's SBUF load waits until g12 on purpose: dense-h0 matmuls
            # must not be schedulable early, or they head-block the in-order
            # PE stream on the AllToAll rendezvous (seen as a 30us mid-loop
            # stall when scheduled at g8).
            if g == 12 or g == NG - 1:
                hf = 0 if g == 12 else 1
                yh = denp.tile([128, KT, 512], dt.float16, name=f"ysb{hf}",
                               tag="ysb")
                yr = y_recv[hf][:].rearrange("b (t p) s -> p (b t) s", p=128)
                for sb_i in range(4):
                    nc.gpsimd.dma_start(
                        out=yh[:, :, sb_i * 128:(sb_i + 1) * 128],
                        in_=yr[:, :, sb_i * 128:(sb_i + 1) * 128])
                ysb[hf] = yh

        # ---- dense ------------------------------------------------------
        # half 0 first (its AllToAll finished back at group 7); the four
        # half-0 passes hide AllToAll #2 + the ysb1 load.
        for i, (cq, half) in enumerate(dense_order):
            for sb_i in range(4):
                acc = psp.tile([128, 512], dt.float32,
                               name=f"dacc{cq}_{half}_{sb_i}", tag="ps")
                for t in range(KT):
                    nc.tensor.matmul(
                        acc[:], ysb[half][:, t, sb_i * 128:(sb_i + 1) * 128],
                        dwq_tiles[i][:, t, :],
                        start=(t == 0), stop=(t == KT - 1))
                ot = outp.tile([128, 512], dt.float32,
                               name=f"ot{cq}_{half}_{sb_i}", tag="ot")
                nc.scalar.activation(ot[:], acc[:], AF.Copy)
                srow = half * 512 + sb_i * 128
                nc.sync.dma_start(
                    out=out_d[srow:srow + 128, cq * 512:(cq + 1) * 512],
                    in_=ot[:])
            if i + 2 < len(dense_order):
                load_dwq(i + 2)
        ctx.close()

    nc.compile()
    return nc


def _stage(hidden_states, positions, qkv_w, qkv_b, q_norm_w, k_norm_w,
           g_w, g_norm_w, dense_w):
    f32 = np.float32
    f16 = np.float16
    hidden_states = np.asarray(hidden_states, dtype=f32)
    positions = np.asarray(positions)
    qkv_w = np.asarray(qkv_w, dtype=f32)
    qkv_b = np.asarray(qkv_b, dtype=f32)
    q_norm_w = np.asarray(q_norm_w, dtype=f32)
    k_norm_w = np.asarray(k_norm_w, dtype=f32)
    g_w = np.asarray(g_w, dtype=f32)
    g_norm_w = np.asarray(g_norm_w, dtype=f32)
    dense_w = np.asarray(dense_w, dtype=f32)
    slopes = _build_slopes()

    hsT = np.ascontiguousarray(hidden_states.T).astype(f16)

    inv_freq = 1.0 / (ROPE_THETA ** (np.arange(0, D, 2, dtype=f32) / D))
    freqs = positions.astype(f32)[:, None] * inv_freq[None, :]  # [S, 64]
    cos = np.cos(freqs).T.astype(f32)     # [64, S]
    sin = np.sin(freqs).T.astype(f32)
    cosf = np.ascontiguousarray(np.concatenate([cos, cos], axis=0)).astype(f16)
    sinf = np.ascontiguousarray(np.concatenate([-sin, sin], axis=0)).astype(f16)

    idx = np.arange(BLK, dtype=f32)
    dwT = np.ascontiguousarray(dense_w.T).astype(f16)
    ones128 = np.ones((128, 1), dtype=f16)
    onesr = np.ones((1, 128), dtype=f16)
    idm = np.eye(128, dtype=f16)
    idsw = np.zeros((128, 128), dtype=f16)
    for m in range(128):
        idsw[(m + 64) % 128, m] = 1.0
    qnw = (q_norm_w.reshape(128, 1) * SCALE).astype(f32)  # SCALE folded in
    knw = k_norm_w.reshape(128, 1).astype(f32).copy()

    in_maps = []
    for j in range(N_CORES):
        heads = [j * HPC + h for h in range(HPC)]
        c0 = j * CPC
        wBm = np.zeros((HID, 768), dtype=f16)
        wBm[:, 0:256] = qkv_w[c0:c0 + CPC, :].T.astype(f16)
        wBm[:, 256:512] = qkv_w[HID + c0:HID + c0 + CPC, :].T.astype(f16)
        wBm[:, 512:768] = g_w[c0:c0 + CPC, :].T.astype(f16)
        wvm = qkv_w[2 * HID + c0:2 * HID + c0 + CPC, :].T.astype(f16)
        wvm = np.ascontiguousarray(wvm)

        qb = qkv_b[c0:c0 + CPC].reshape(HPC, 128).T       # [128, 2]
        kb = qkv_b[HID + c0:HID + c0 + CPC].reshape(HPC, 128).T
        qkbm = np.ascontiguousarray(
            np.concatenate([qb, kb], axis=1)).astype(f32)  # [128, 4]
        vb = qkv_b[2 * HID + c0:2 * HID + c0 + CPC].astype(f16)  # [256]
        vbm = np.ascontiguousarray(np.broadcast_to(vb[None, :], (128, 256)))

        sl = slopes[heads]  # [HPC]
        qdec = np.exp(-sl[:, None] * (idx + 1.0)[None, :]).astype(f16)
        qdec = np.ascontiguousarray(
            np.broadcast_to(qdec[None, :, :], (128, HPC, BLK)))
        kd = np.exp(-sl[:, None] * (BLK - 1.0 - idx)[None, :]).astype(f16)
        kdecm = np.ascontiguousarray(
            np.broadcast_to(kd[None, :, :], (128, HPC, BLK)))
        dif = idx[:, None] - idx[None, :]           # [i, j]
        diagT = np.zeros((128, HPC, 2, BLK), dtype=f16)
        for hh in range(HPC):
            dd = np.where(
                dif >= 0,
                np.exp(-sl[hh] * np.where(dif >= 0, dif, 0.0)),
                0.0).astype(f16)                    # [i, j]
            ddT = dd.T                               # [j, i]
            diagT[:, hh, 0, :] = ddT[0:128]
            diagT[:, hh, 1, :] = ddT[128:256]
        blkdec = np.ascontiguousarray(np.broadcast_to(
            np.exp(-sl * BLK).astype(f32)[None, :], (128, HPC)))
        gnwm = np.ascontiguousarray(
            g_norm_w[c0:c0 + CPC].reshape(HPC, 128).T).astype(f32)

        in_maps.append({
            "hsT": hsT, "wB": wBm, "wv": wvm, "dwT": dwT,
            "cosf": cosf, "sinf": sinf,
            "qdec": qdec, "kdec": kdecm, "diagT": diagT,
            "qkb": qkbm, "vb": vbm,
            "qnw": qnw, "knw": knw, "gnw": gnwm, "blkdec": blkdec,
            "ones128": ones128, "onesr": onesr,
            "idm": idm, "idsw": idsw,
            "zkv": np.zeros((128, 128), dtype=f16),
        })
    return in_maps


def kernel(**inputs):
    from concourse.bass_utils import run_bass_kernel_spmd

    if "nc" not in _cache:
        _cache["nc"] = _build_program()
    nc = _cache["nc"]
    in_maps = _stage(**inputs)
    res = run_bass_kernel_spmd(nc, in_maps, list(range(N_CORES)))
    return _assemble(res)


def _assemble(res):
    # core j's shard is rows [4096*p + 512*j, +512) for p in {0,1}
    out = np.empty((S, HID), dtype=np.float32)
    for j in range(N_CORES):
        rj = res.results[j]["out"]
        for p in range(2):
            out[4096 * p + 512 * j:4096 * p + 512 * (j + 1)] = \
                rj[512 * p:512 * (p + 1)]
    return out
